# revision 4
# baseline (speedup 1.0000x reference)
"""AttentionPooler Trainium2 kernel (8 NeuronCores, data-parallel over batch).

Reference computation (layer 7 of hidden_states, N=16, L=512, D=768, H=256,
S=1024 spans):
    proj   = hs @ W_in + b_in            # (N, L, H)
    scores = proj @ w_score              # (N, L)
    att    = softmax(scores masked to each span)
    out[s] = sum_l att[s,l] * proj[idx_s, l]

Sharding: core c owns batches [2c, 2c+2) -> 1024 rows of hs. Spans are routed
host-side to the core owning their batch. Per core the device computes, in
bf16 on the TensorEngine:
    projS = hsT.T @ [W_in | v]  with v = W_in @ w_score  (scores ride along as
                                 column H; softmax is shift-invariant so the
                                 b_in contribution to scores cancels)
    E     = exp(scores)          (no max-subtraction needed: |scores| < ~1)
    G     = E * maskT            (maskT built host-side from span bounds)
    U     = G.T @ [proj | 1]     (column H = sum of weights Z)
    out   = U[:, :H] / U[:, H] (+ b_in)
"""

import sys

sys.path.insert(0, "/opt/trn_rl_repo")

import numpy as np
import ml_dtypes

LAYER = 7
N, L, D, H, S = 16, 512, 768, 256, 1024
NCORES = 8
NB = N // NCORES          # batches per core
R = NB * L                # rows per core
KD = D // 128             # contraction chunks (6)
RM = R // 128             # row chunks (8)
BF16 = ml_dtypes.bfloat16


def _split_waits(nc):
    """This walrus build rejects instructions carrying >1 semaphore wait
    ("Too many sync wait commands"). Tile attaches multi-waits freely, so
    split them: hoist all but the last wait onto standalone NoOps on the
    same engine immediately before the instruction."""
    from concourse import mybir

    for fn in nc.m.functions:
        for bb in fn.blocks:
            insts = list(bb.instructions)
            new = []
            changed = False
            for ins in insts:
                si = ins.sync_info
                waits = list(si.on_wait) if si is not None else []
                if len(waits) > 1:
                    changed = True
                    for i, w in enumerate(waits[:-1]):
                        nop = mybir.InstNoOp(name=f"{ins.name}-sw{i}")
                        nop.engine = ins.engine
                        nop.sync_info = mybir.SyncInfo(on_wait=[w], on_update=[])
                        new.append(nop)
                    ins.sync_info = mybir.SyncInfo(
                        on_wait=[waits[-1]], on_update=list(si.on_update)
                    )
                new.append(ins)
            if changed:
                bb.instructions = new


def _build_graph(SP, with_bias):
    import concourse.bass as bass
    import concourse.tile as tile
    from concourse import mybir

    bf = mybir.dt.bfloat16
    f32 = mybir.dt.float32

    nc = bass.Bass()
    hsT = nc.declare_dram_parameter("hsT", [D, R], bf, isOutput=False)
    maskT = nc.declare_dram_parameter("maskT", [R, SP], bf, isOutput=False)
    W = nc.declare_dram_parameter("W", [D, H + 1], bf, isOutput=False)
    if with_bias:
        brep = nc.declare_dram_parameter("brep", [128, H], f32, isOutput=False)
    out = nc.declare_dram_parameter("out", [SP, H], f32, isOutput=True)

    sp_chunks = []
    o = 0
    while o < SP:
        sp_chunks.append((o, min(128, SP - o)))
        o += 128

    with tile.TileContext(nc) as tc:
        with (
            tc.tile_pool(name="consts", bufs=1) as consts,
            tc.tile_pool(name="hs", bufs=1) as hs_pool,
            tc.tile_pool(name="projsb", bufs=1) as proj_pool,
            tc.tile_pool(name="gp", bufs=1) as g_pool,
            tc.tile_pool(name="stat", bufs=1) as stat_pool,
            tc.tile_pool(name="outp", bufs=1) as out_pool,
            tc.tile_pool(name="psA", bufs=1, space="PSUM") as psA,
            tc.tile_pool(name="psU", bufs=1, space="PSUM") as psU,
        ):
            # --- loads (HWDGE). W first (gates all matmuls), then hs chunks,
            # mask last (only needed by the G stage).
            w_tile = consts.tile([128, KD, H + 1], bf, tag="w", name="w")
            nc.sync.dma_start(out=w_tile, in_=W[:, :].rearrange("(k p) n -> p k n", p=128))

            hsT_r = hsT[:, :].rearrange("(k p) r -> p k r", p=128)
            hs_tiles = []
            for i in range(3):
                t = hs_pool.tile([128, 2, R], bf, tag=f"hs{i}", name=f"hs{i}")
                nc.sync.dma_start(out=t, in_=hsT_r[:, 2 * i : 2 * i + 2, :])
                hs_tiles.append(t)

            mask_tile = consts.tile([128, RM, SP], bf, tag="mask", name="mask")
            nc.sync.dma_start(
                out=mask_tile, in_=maskT[:, :].rearrange("(m p) s -> p m s", p=128)
            )

            if with_bias:
                b_tile = consts.tile([128, H], f32, tag="b", name="b")
                nc.sync.dma_start(out=b_tile, in_=brep[:, :])

            U_tiles = [
                psU.tile([128, H + 1], f32, tag=f"U{j}", name=f"U{j}") for j in range(len(sp_chunks))
            ]

            # Row-chunk groups of 4 so proj psum (4 banks) + U psum (<=2 banks)
            # fit in the 8 PSUM banks.
            for g0 in range(0, RM, 4):
                ms = range(g0, g0 + 4)
                ps = {m: psA.tile([128, H + 1], f32, tag=f"proj{m % 4}", name=f"proj{m}") for m in ms}
                for k in range(KD):
                    for m in ms:
                        nc.tensor.matmul(
                            ps[m],
                            lhsT=hs_tiles[k // 2][:, k % 2, m * 128 : (m + 1) * 128],
                            rhs=w_tile[:, k, :],
                            start=(k == 0),
                            stop=(k == KD - 1),
                        )
                for m in ms:
                    # E = exp(scores) straight off PSUM (ACT engine)
                    e_m = stat_pool.tile([128, 1], f32, tag=f"e{m}", name=f"e{m}")
                    nc.scalar.activation(
                        out=e_m,
                        in_=ps[m][:, H : H + 1],
                        func=mybir.ActivationFunctionType.Exp,
                    )
                    # proj -> SBUF bf16 (+ ones column for the Z sum)
                    psb = proj_pool.tile([128, H + 1], bf, tag=f"psb{m}", name=f"psb{m}")
                    nc.vector.tensor_copy(out=psb[:, 0:H], in_=ps[m][:, 0:H])
                    nc.gpsimd.memset(psb[:, H : H + 1], 1.0)
                    # G = E * maskT   (bf16, per-partition scalar multiply)
                    g_m = g_pool.tile([128, SP], bf, tag=f"g{m}", name=f"gt{m}")
                    nc.vector.tensor_scalar_mul(
                        out=g_m, in0=mask_tile[:, m, :], scalar1=e_m
                    )
                    for j, (so, sn) in enumerate(sp_chunks):
                        nc.tensor.matmul(
                            U_tiles[j][:sn],
                            lhsT=g_m[:, so : so + sn],
                            rhs=psb[:, :],
                            start=(m == 0),
                            stop=(m == RM - 1),
                        )

            for j, (so, sn) in enumerate(sp_chunks):
                rc = stat_pool.tile([128, 1], f32, tag=f"rc{j}", name=f"rc{j}")
                nc.vector.reciprocal(out=rc[:sn], in_=U_tiles[j][:sn, H : H + 1])
                pooled = out_pool.tile([128, H], f32, tag=f"pool{j}", name=f"pool{j}")
                nc.vector.tensor_scalar_mul(
                    out=pooled[:sn], in0=U_tiles[j][:sn, 0:H], scalar1=rc[:sn]
                )
                if with_bias:
                    nc.vector.tensor_add(
                        out=pooled[:sn], in0=pooled[:sn], in1=b_tile[:sn]
                    )
                nc.sync.dma_start(out=out[so : so + sn, :], in_=pooled[:sn])

    _split_waits(nc)
    return nc


def _prepare(inputs):
    hs7 = np.asarray(inputs["hidden_states"])[LAYER]          # (N, L, D) f32
    spans = np.asarray(inputs["target_spans"])                # (S, 3) int32
    W_in = np.asarray(inputs["W_in"], dtype=np.float32)
    b_in = np.asarray(inputs["b_in"], dtype=np.float32)
    w_score = np.asarray(inputs["w_score"], dtype=np.float32)

    idx, a, b = spans[:, 0], spans[:, 1], spans[:, 2]
    core_of = idx // NB
    sels = [np.nonzero(core_of == c)[0] for c in range(NCORES)]
    max_cnt = max(len(s) for s in sels)
    SP = max(32, -(-max_cnt // 32) * 32)

    v = W_in @ w_score                                        # (D,)
    W_aug = np.concatenate([W_in, v[:, None]], axis=1).astype(BF16)
    with_bias = bool(np.any(b_in))

    in_maps = []
    for c in range(NCORES):
        hs_c = hs7[c * NB : (c + 1) * NB].reshape(R, D)
        hsT_c = np.ascontiguousarray(hs_c.T).astype(BF16)     # (D, R)
        sel = sels[c]
        m_c = len(sel)
        mask = np.zeros((R, SP), dtype=BF16)
        li = idx[sel] - c * NB
        rs = li * L + a[sel]
        re = li * L + b[sel]
        for j in range(m_c):
            mask[rs[j] : re[j], j] = 1
        mask[0, m_c:] = 1                                     # dummy pad spans
        m = {"hsT": hsT_c, "maskT": mask, "W": W_aug}
        if with_bias:
            m["brep"] = np.broadcast_to(b_in, (128, H)).copy()
        in_maps.append(m)
    return SP, with_bias, in_maps, sels


def _run(inputs, trace=False, **kw):
    from concourse.bass_utils import run_bass_kernel_spmd

    SP, with_bias, in_maps, sels = _prepare(inputs)
    nc = _build_graph(SP, with_bias)
    res = run_bass_kernel_spmd(
        nc, in_maps, core_ids=list(range(NCORES)), trace=trace, **kw
    )
    out_full = np.zeros((S, H), dtype=np.float32)
    for c in range(NCORES):
        sel = sels[c]
        out_full[sel] = res.results[c]["out"][: len(sel)]
    return out_full, res


def kernel(**inputs):
    return _run(inputs, trace=False)[0]
